# revision 16
# baseline (speedup 1.0000x reference)
"""Trainium2 Bass kernel for nn_CharCondDiscriminatorAP (moe_routing).

Strategy (8 NeuronCores, SPMD):
- Launch 1 (backbone): data-parallel over (batch=2) x (W quarters=4).
  Each core runs the full conv stack on a W-slab with halos.  All per-core
  differences (slab data, folded style biases, edge masks, style edge
  corrections, sal slabs) are host-staged inputs so one program serves all
  cores.  Style contributions are folded into per-channel biases; pooling is
  implemented as 2x2 sums with the 1/4 folded into the next conv's weights.
  Outputs: per-core mL slice [256,3,16] and pM slice [1,16].
- Host routing: tokens grouped by class (expert), experts bin-packed onto
  cores, patch windows gathered from mL into a canonical matmul-ready layout.
- Launch 2 (experts): each core holds ~1/8 of the used CharDisc weight banks
  (bf16) and computes conv+fc for its tokens' patches.
"""
import sys
sys.path.insert(0, '/opt/trn_rl_repo')
sys.path.insert(0, '/root/.axon_site')
import numpy as np
import ml_dtypes

import bass_rust
import concourse.bass as bass
import concourse.mybir as mybir
from concourse.bass import AP
from concourse.bass_utils import run_bass_kernel_spmd
from concourse.tile import TileContext
from concourse.masks import make_identity

F32 = mybir.dt.float32
F32R = mybir.dt.float32r
BF16 = mybir.dt.bfloat16
ALU = mybir.AluOpType
ACT = mybir.ActivationFunctionType
LEAK = 0.1
BF = ml_dtypes.bfloat16

N_CORES = 8

# backbone slab geometry: level -> (width, h_rows, lo(q), true_extent, mask_win)
# lo(q) given as (mult, off): lo = mult*q + off
GEOM = {
    'h':  (194, 58, (128, -33), 512, 33),
    'g':  (192, 56, (128, -32), 512, 32),
    'hp': (96,  28, (64, -16), 256, 0),
    'm':  (94,  26, (64, -15), 256, 15),
    'g2': (92,  24, (64, -14), 256, 14),
    'mL': (46,  12, (32, -7), 128, 0),
    'g3': (44,  10, (32, -6), 128, 6),
    'mL2': (22, 5,  (16, -3), 64, 0),
    'g4': (20,  3,  (16, -2), 64, 2),
}


class PatchedTileContext(TileContext):
    """This container's walrus allows only ONE sync-wait per instruction on
    several opcodes; stock Tile attaches one wait per dependency processor.
    Post-pass: hoist excess waits onto injected same-engine NoOps placed
    immediately before the instruction (sequencers execute in order, so
    waiting on the nops first is equivalent)."""

    MAX_WAITS = 1

    def __exit__(self, *args):
        res = super().__exit__(*args)
        self._split_excess_waits()
        return res

    def _split_excess_waits(self):
        nc = self.nc
        nop_id = [0]

        def fix_block(bb):
            insts = list(bb.instructions)
            out = []
            changed = False
            for inst in insts:
                si = inst.sync_info
                if si is not None and len(si.on_wait) > self.MAX_WAITS:
                    waits = list(si.on_wait)
                    keep = waits[-self.MAX_WAITS:]
                    hoist = waits[:-self.MAX_WAITS]
                    for w in hoist:
                        nop = mybir.InstNoOp(
                            name=f"I-waitfix-{nop_id[0]}", ins=[], outs=[])
                        nop_id[0] += 1
                        nop.engine = inst.engine
                        nop.sync_info = bass_rust.SyncInfo(
                            on_wait=[w], on_update=[])
                        out.append(nop)
                    inst.sync_info = bass_rust.SyncInfo(
                        on_wait=keep, on_update=list(si.on_update))
                    changed = True
                out.append(inst)
            if changed:
                try:
                    bb.instructions = out
                except Exception:
                    bb.instructions.clear()
                    bb.instructions.extend(out)

        for fn in nc.m.functions:
            for bb in fn.blocks:
                fix_block(bb)

    def _drain_and_barrier(self, tick_clock, wait_clock):
        nc = self.nc
        gc = tick_clock.global_clock
        nz = [(p, t) for p, t in enumerate(list(gc)) if t > 0]
        for p, t in nz[:-1]:
            v = bass_rust.VectorClock()
            v.require_at_least(p, t)
            nop = nc.sync.nop()
            wait_clock.add_sem_waits(nop.ins, bass_rust.ScopedClock({None: v}))
        drain_inst = nc.sync.drain()
        if nz:
            p, t = nz[-1]
            v = bass_rust.VectorClock()
            v.require_at_least(p, t)
            wait_clock.add_sem_waits(drain_inst.ins, bass_rust.ScopedClock({None: v}))
        nc.all_engine_barrier()
        assert self.sems is not None
        popped = nc._tile_sem_poison_stack.pop()
        assert popped is self._sem_poison
        nc.clear_and_free_semaphores(list(self.sems.allocated().values()))
        nc.all_engine_barrier()


def _bcast_h(ap2d, h):
    """Insert a step-0 (broadcast) dim of count h between partition and free."""
    dims = [list(d) for d in ap2d.ap]
    assert len(dims) == 2
    return AP(ap2d.tensor, ap2d.offset, [dims[0], [0, h], dims[1]])


# ---------------------------------------------------------------------------
# Backbone kernel builder
# ---------------------------------------------------------------------------

def build_backbone():
    nc = bass.Bass()
    dram = {}

    def din(name, shape, dt=F32):
        dram[name] = nc.dram_tensor(name, shape, dt, kind="ExternalInput")
        return dram[name]

    x49 = din("x49", [49, 58 * 194], BF16)
    w1 = din("w1", [49, 64], BF16)
    b1 = din("b1", [64, 1])
    w2 = din("w2", [64, 9 * 64])
    b2 = din("b2", [64, 1])
    cw2 = din("cw2", [64, 192])      # corr for L2 (full width, mostly zero)
    w3 = din("w3", [64, 9 * 128])
    b3 = din("b3", [128, 1])
    w4 = din("w4", [128, 9 * 128])
    b4 = din("b4", [128, 1])
    cw4 = din("cw4", [128, 92])
    w5 = din("w5", [128, 9 * 128])
    b5 = din("b5", [128, 1])
    wsal = din("wsal", [112, 3 * 128])
    sal = din("sal", [112, 46])
    w6 = din("w6", [128, 18 * 128])
    b6 = din("b6", [128, 2])
    wfm = din("wfm", [128, 18])
    bfm = din("bfm", [1, 1])
    # per-level edge masks [P, 2*win] (left win cols | right win cols)
    mk = {}
    for lv in ('h', 'g', 'm', 'g2', 'g3', 'g4'):
        w_, hh, _, _, win = GEOM[lv]
        p = 64 if lv in ('h', 'g') else 128
        mk[lv] = din(f"mask_{lv}", [p, 2 * win])

    mL_out = nc.dram_tensor("mL_out", [2, 128, 3, 16], F32, kind="ExternalOutput")
    pM_out = nc.dram_tensor("pM_out", [1, 16], F32, kind="ExternalOutput")

    def r(ap):
        return ap if ap.dtype == F32R else ap.bitcast(F32R)

    with PatchedTileContext(nc) as tc:
        with tc.tile_pool(name="wt", bufs=1) as wt, \
             tc.tile_pool(name="act", bufs=1) as actp, \
             tc.tile_pool(name="ps", bufs=4, space="PSUM") as ps:

            # ---- load weights/support tensors
            t_x49 = actp.tile([49, 58 * 194], BF16, tag="bigA")
            nc.sync.dma_start(t_x49[:, :], x49[:, :])
            t_w1 = wt.tile([49, 64], BF16)
            nc.sync.dma_start(t_w1[:, :], w1[:, :])
            t_b1 = wt.tile([64, 1], F32)
            nc.sync.dma_start(t_b1[:, :], b1[:, :])
            def round_load(dst, dram_ap):
                stg = wt.tile(list(dst.shape), F32, name="wstage", tag="wstage")
                nc.sync.dma_start(stg[:, :], dram_ap)
                nc.vector.tensor_copy(dst[:, :], stg[:, :])

            t_w2 = wt.tile([64, 9 * 64], F32R)
            round_load(t_w2, w2[:, :])
            t_b2 = wt.tile([64, 1], F32)
            nc.sync.dma_start(t_b2[:, :], b2[:, :])
            t_cw2 = wt.tile([64, 192], F32)
            nc.sync.dma_start(t_cw2[:, :], cw2[:, :])
            t_w3 = wt.tile([64, 9 * 128], F32R)
            round_load(t_w3, w3[:, :])
            t_b3 = wt.tile([128, 1], F32)
            nc.sync.dma_start(t_b3[:, :], b3[:, :])
            t_w4 = wt.tile([128, 9 * 128], F32R)
            round_load(t_w4, w4[:, :])
            t_b4 = wt.tile([128, 1], F32)
            nc.sync.dma_start(t_b4[:, :], b4[:, :])
            t_cw4 = wt.tile([128, 92], F32)
            nc.sync.dma_start(t_cw4[:, :], cw4[:, :])
            t_w5 = wt.tile([128, 9 * 128], F32R)
            round_load(t_w5, w5[:, :])
            t_b5 = wt.tile([128, 1], F32)
            nc.sync.dma_start(t_b5[:, :], b5[:, :])
            t_wsal = wt.tile([112, 3 * 128], F32R)
            round_load(t_wsal, wsal[:, :])
            t_sal = wt.tile([112, 46], F32)
            nc.sync.dma_start(t_sal[:, :], sal[:, :])
            t_w6 = wt.tile([128, 18 * 128], F32R)
            round_load(t_w6, w6[:, :])
            t_b6 = wt.tile([128, 2], F32)
            nc.sync.dma_start(t_b6[:, :], b6[:, :])
            t_wfm = wt.tile([128, 18], F32R)
            round_load(t_wfm, wfm[:, :])
            t_bfm = wt.tile([1, 1], F32)
            nc.sync.dma_start(t_bfm[:, :], bfm[:, :])
            t_alpha = wt.tile([128, 1], F32)
            nc.vector.memset(t_alpha[:, :], LEAK)
            t_mk = {}
            for lv in ('h', 'g', 'm', 'g2', 'g3', 'g4'):
                _, _, _, _, win = GEOM[lv]
                p = 64 if lv in ('h', 'g') else 128
                t_mk[lv] = wt.tile([p, 2 * win], F32, name=f"mask{lv}",
                                   tag=f"mask{lv}")
                nc.sync.dma_start(t_mk[lv][:, :], mk[lv][:, :])

            def apply_mask(slab, lv, P):
                w_, hh, _, _, win = GEOM[lv]
                s3 = slab[:, :].rearrange("p (h w) -> p h w", h=hh)
                # left window
                nc.vector.tensor_tensor(
                    s3[:, :, :win], s3[:, :, :win],
                    _bcast_h(t_mk[lv][:P, :win], hh), op=ALU.mult)
                nc.vector.tensor_tensor(
                    s3[:, :, w_ - win:], s3[:, :, w_ - win:],
                    _bcast_h(t_mk[lv][:P, win:], hh), op=ALU.mult)

            # ---- L1: conv1 7x7 (K=49) -> h [64, 58*194]
            t_h = actp.tile([64, 58 * 194], F32R, tag="bigB")
            for i in range(29):
                off = 2 * i * 194
                pt = ps.tile([64, 388], F32, tag="pp")
                nc.tensor.matmul(pt[:, :], t_w1[:, :], t_x49[:, off:off + 388],
                                 start=True, stop=True)
                nc.scalar.activation(t_h[:, off:off + 388], pt[:, :], ACT.Prelu,
                                     bias=t_b1[:, :1], alpha=t_alpha[:64, :1])
            apply_mask(t_h, 'h', 64)

            # ---- L2: c1w1 3x3 (K=64, folded style) -> g [64, 56*192]
            t_g = actp.tile([64, 56 * 192], F32R, tag="bigC")
            for i in range(28):
                rows = 2
                pt = ps.tile([64, 384], F32, tag="pp")
                for k in range(9):
                    dh, dw = divmod(k, 3)
                    rhs = AP(t_h.tensor, t_h[:, :].offset + (2 * i + dh) * 194 + dw,
                             [list(t_h[:, :].ap[0]), [194, rows], [1, 192]])
                    nc.tensor.matmul(pt[:, :], r(t_w2[:, k * 64:(k + 1) * 64]), r(rhs),
                                     start=(k == 0), stop=(k == 8))
                # style edge correction (pre-lrelu), broadcast over the 2 rows
                pt3 = pt[:, :].rearrange("p (h w) -> p h w", h=rows)
                nc.vector.tensor_tensor(pt3[:, :, :], pt3[:, :, :],
                                        _bcast_h(t_cw2[:, :], rows), op=ALU.add)
                nc.scalar.activation(t_g[:, i * 2 * 192:(i + 1) * 2 * 192], pt[:, :],
                                     ACT.Prelu, bias=t_b2[:, :1], alpha=t_alpha[:64, :1])
            apply_mask(t_g, 'g', 64)

            # ---- pool g -> hp [64, 28*96] (sum; /4 folded into w3)
            t_gt = actp.tile([64, 56 * 96], F32R, tag="bigA")  # w-paired
            g3d = t_g[:, :].rearrange("p (a two) -> p a two", two=2)
            nc.vector.tensor_tensor(t_gt[:, :], g3d[:, :, 0], g3d[:, :, 1], op=ALU.add)
            t_hp = actp.tile([64, 28 * 96], F32R, tag="bigD")
            gt3 = t_gt[:, :].rearrange("p (h two w) -> p h two w", two=2, w=96)
            nc.vector.tensor_tensor(t_hp[:, :].rearrange("p (h w) -> p h w", w=96),
                                    gt3[:, :, 0, :], gt3[:, :, 1, :], op=ALU.add)

            # ---- L3: c1w2 (K=64) -> m [128, 26*94]
            t_m = actp.tile([128, 26 * 94], F32R, tag="bigB")
            r0 = 0
            for i in range(7):
                rows = min(4, 26 - r0)
                pt = ps.tile([128, 4 * 94], F32, tag="pp")
                for k in range(9):
                    dh, dw = divmod(k, 3)
                    rhs = AP(t_hp.tensor, t_hp[:, :].offset + (r0 + dh) * 96 + dw,
                             [list(t_hp[:, :].ap[0]), [96, rows], [1, 94]])
                    nc.tensor.matmul(pt[:, :rows * 94], r(t_w3[:, k * 128:(k + 1) * 128]), r(rhs),
                                     start=(k == 0), stop=(k == 8))
                nc.scalar.activation(t_m[:, r0 * 94:(r0 + rows) * 94],
                                     pt[:, :rows * 94],
                                     ACT.Prelu, bias=t_b3[:, :1], alpha=t_alpha[:, :1])
                r0 += rows
            apply_mask(t_m, 'm', 128)

            # ---- L4: c2w (K=128, folded style2) -> g2 [128, 24*92]
            t_g2 = actp.tile([128, 24 * 92], F32R, tag="bigC")
            for i in range(6):
                rows = 4
                pt = ps.tile([128, 4 * 92], F32, tag="pp")
                for k in range(9):
                    dh, dw = divmod(k, 3)
                    rhs = AP(t_m.tensor, t_m[:, :].offset + (4 * i + dh) * 94 + dw,
                             [list(t_m[:, :].ap[0]), [94, rows], [1, 92]])
                    nc.tensor.matmul(pt[:, :], r(t_w4[:, k * 128:(k + 1) * 128]), r(rhs),
                                     start=(k == 0), stop=(k == 8))
                pt3 = pt[:, :].rearrange("p (h w) -> p h w", h=rows)
                nc.vector.tensor_tensor(pt3[:, :, :], pt3[:, :, :],
                                        _bcast_h(t_cw4[:, :], rows), op=ALU.add)
                nc.scalar.activation(t_g2[:, i * 4 * 92:(i + 1) * 4 * 92], pt[:, :],
                                     ACT.Prelu, bias=t_b4[:, :1], alpha=t_alpha[:, :1])
            apply_mask(t_g2, 'g2', 128)

            # ---- pool g2 -> mL [128, 12*46]
            t_g2t = actp.tile([128, 24 * 46], F32R, tag="bigA")
            g23 = t_g2[:, :].rearrange("p (a two) -> p a two", two=2)
            nc.vector.tensor_tensor(t_g2t[:, :], g23[:, :, 0], g23[:, :, 1], op=ALU.add)
            t_mL = actp.tile([128, 12 * 46], F32R)
            g2t3 = t_g2t[:, :].rearrange("p (h two w) -> p h two w", two=2, w=46)
            nc.vector.tensor_tensor(t_mL[:, :].rearrange("p (h w) -> p h w", w=46),
                                    g2t3[:, :, 0, :], g2t3[:, :, 1, :], op=ALU.add)

            # ---- L5: c3w1 (K=128 main + K=112 sal) -> g3 [128, 10*44]
            t_salb = actp.tile([112, 10 * 46], F32R)
            nc.vector.tensor_copy(
                t_salb[:, :].rearrange("p (h w) -> p h w", w=46),
                _bcast_h(t_sal[:, :], 10))
            t_g3 = actp.tile([128, 10 * 44], F32R)
            pt = ps.tile([128, 440], F32, tag="pp")
            for k in range(9):
                dh, dw = divmod(k, 3)
                rhs = AP(t_mL.tensor, t_mL[:, :].offset + dh * 46 + dw,
                         [list(t_mL[:, :].ap[0]), [46, 10], [1, 44]])
                nc.tensor.matmul(pt[:, :], r(t_w5[:, k * 128:(k + 1) * 128]), r(rhs),
                                 start=(k == 0), stop=False)
            for dw in range(3):
                rhs = AP(t_salb.tensor, t_salb[:, :].offset + dw,
                         [list(t_salb[:, :].ap[0]), [46, 10], [1, 44]])
                nc.tensor.matmul(pt[:, :], r(t_wsal[:, dw * 128:(dw + 1) * 128]), r(rhs),
                                 start=False, stop=(dw == 2))
            nc.scalar.activation(t_g3[:, :], pt[:, :], ACT.Prelu,
                                 bias=t_b5[:, :1], alpha=t_alpha[:, :1])
            apply_mask(t_g3, 'g3', 128)

            # ---- pool g3 -> mL2 [128, 5*22]
            t_g3t = actp.tile([128, 10 * 22], F32R, tag="bigD")
            g33 = t_g3[:, :].rearrange("p (a two) -> p a two", two=2)
            nc.vector.tensor_tensor(t_g3t[:, :], g33[:, :, 0], g33[:, :, 1], op=ALU.add)
            t_mL2 = actp.tile([128, 5 * 22], F32R, tag="bigB")
            g3t3 = t_g3t[:, :].rearrange("p (h two w) -> p h two w", two=2, w=22)
            nc.vector.tensor_tensor(t_mL2[:, :].rearrange("p (h w) -> p h w", w=22),
                                    g3t3[:, :, 0, :], g3t3[:, :, 1, :], op=ALU.add)

            # ---- L6: c3w2 (K=128, Cout=256 in 2 chunks) -> g4 [128, 2*3*20]
            t_g4 = actp.tile([128, 2 * 3 * 20], F32R)
            for cb in range(2):
                pt = ps.tile([128, 60], F32, tag="pp")
                for k in range(9):
                    dh, dw = divmod(k, 3)
                    rhs = AP(t_mL2.tensor, t_mL2[:, :].offset + dh * 22 + dw,
                             [list(t_mL2[:, :].ap[0]), [22, 3], [1, 20]])
                    nc.tensor.matmul(pt[:, :], r(t_w6[:, (k * 2 + cb) * 128:(k * 2 + cb) * 128 + 128]), r(rhs),
                                     start=(k == 0), stop=(k == 8))
                nc.scalar.activation(t_g4[:, cb * 60:(cb + 1) * 60], pt[:, :],
                                     ACT.Prelu, bias=t_b6[:, cb:cb + 1], alpha=t_alpha[:, :1])
            # mask g4: left/right 2 cols of each (cb, r) row-block
            win = 2
            g44 = t_g4[:, :].rearrange("p (cr w) -> p cr w", w=20)
            mml = AP(t_mk['g4'].tensor, t_mk['g4'][:, :].offset,
                     [list(t_mk['g4'][:, :].ap[0]), [0, 6], [1, win]])
            mmr = AP(t_mk['g4'].tensor, t_mk['g4'][:, :].offset + win,
                     [list(t_mk['g4'][:, :].ap[0]), [0, 6], [1, win]])
            nc.vector.tensor_tensor(g44[:, :, :win], g44[:, :, :win], mml, op=ALU.mult)
            nc.vector.tensor_tensor(g44[:, :, 20 - win:], g44[:, :, 20 - win:],
                                    mmr, op=ALU.mult)

            # ---- fm conv -> pM [1, 16]
            ptf = ps.tile([1, 16], F32, tag="pp")
            kk = 0
            for cb in range(2):
                for k in range(9):
                    dh, dw = divmod(k, 3)
                    rhs = t_g4[:, cb * 60 + dh * 20 + 1 + dw:
                               cb * 60 + dh * 20 + 1 + dw + 16]
                    nc.tensor.matmul(ptf[:, :], r(t_wfm[:, cb * 9 + k:cb * 9 + k + 1]), r(rhs),
                                     start=(kk == 0), stop=(kk == 17))
                    kk += 1
            t_pM = actp.tile([1, 16], F32)
            nc.scalar.activation(t_pM[:, :], ptf[:, :], ACT.Identity,
                                 bias=t_bfm[:1, :1])
            nc.sync.dma_start(pM_out[:, :], t_pM[:, :])

            # ---- mL slice out: g4[:, cb, :, 2:18] -> mL_out[cb]
            for cb in range(2):
                src = AP(t_g4.tensor, t_g4[:, :].offset + cb * 60 + 2,
                         [list(t_g4[:, :].ap[0]), [20, 3], [1, 16]]).bitcast(F32)
                nc.sync.dma_start(mL_out[cb, :, :, :], src)

    return nc


# ---------------------------------------------------------------------------
# Expert kernel builder (parameterized by slot capacity profile)
# ---------------------------------------------------------------------------

def build_expert(caps):
    S = len(caps)
    NTOT = int(sum(caps))
    NROWS = NTOT * 5
    pos0 = [int(v) for v in np.concatenate([[0], np.cumsum(caps)])]

    nc = bass.Bass()
    patches = nc.dram_tensor("patches", [128, 18 * 3 * NTOT], BF16,
                             kind="ExternalInput")
    wexp = nc.dram_tensor("wexp", [S, 128, 19 * 128], BF16, kind="ExternalInput")
    wfc1a = nc.dram_tensor("wfc1a", [S, 128, 128], BF16, kind="ExternalInput")
    wfc1b = nc.dram_tensor("wfc1b", [S, 33, 128], BF16, kind="ExternalInput")
    charT = nc.dram_tensor("charT", [32, NTOT], BF16, kind="ExternalInput")
    w2rep = nc.dram_tensor("w2rep", [S, 42, 128], F32, kind="ExternalInput")
    b2rep = nc.dram_tensor("b2rep", [S, 42, 1], F32, kind="ExternalInput")
    pchar_out = nc.dram_tensor("pchar_out", [NTOT, 1], F32, kind="ExternalOutput")

    with PatchedTileContext(nc) as tc:
        with tc.tile_pool(name="sb", bufs=1) as sb, \
             tc.tile_pool(name="wp", bufs=3) as wp, \
             tc.tile_pool(name="ps", bufs=2, space="PSUM") as ps:

            t_p = sb.tile([128, 18 * 3 * NTOT], BF16)
            nc.sync.dma_start(t_p[:, :], patches[:, :])
            t_char = sb.tile([32, NTOT], BF16)
            nc.sync.dma_start(t_char[:, :], charT[:, :])
            t_ones = sb.tile([1, 128], BF16)
            nc.vector.memset(t_ones[:, :], 1.0)
            t_alpha = sb.tile([128, 1], F32)
            nc.vector.memset(t_alpha[:, :], LEAK)
            t_id = sb.tile([128, 128], BF16)
            make_identity(nc, t_id[:, :])

            for s in range(S):
                cap = int(caps[s])
                t_w = wp.tile([128, 19 * 128], BF16, tag="wexp")
                nc.sync.dma_start(t_w[:, :], wexp[s, :, :])
                t_w1a = wp.tile([128, 128], BF16, tag="wfc1a")
                nc.sync.dma_start(t_w1a[:, :], wfc1a[s, :, :])
                t_w1b = wp.tile([32, 128], BF16, tag="wfc1b")
                nc.sync.dma_start(t_w1b[:, :], wfc1b[s, :32, :])
                t_w1c = wp.tile([1, 128], BF16, tag="wfc1c")
                nc.sync.dma_start(t_w1c[:, :], wfc1b[s, 32:33, :])

                # conv: psum [3*cap, 128] over 18 K-chunks + bias row
                pt = ps.tile([3 * cap, 128], F32, tag="psco")
                for k in range(18):
                    base = k * 3 * NTOT + 3 * pos0[s]
                    lhsT = t_p[:, base:base + 3 * cap]
                    nc.tensor.matmul(pt[:, :], lhsT, t_w[:, k * 128:(k + 1) * 128],
                                     start=(k == 0), stop=False)
                nc.tensor.matmul(pt[:, :], t_ones[:1, :3 * cap],
                                 t_w[:1, 18 * 128:19 * 128], start=False, stop=True)
                # lrelu -> bf16
                t_co = sb.tile([3 * 42, 128], BF16, tag="co")
                nc.scalar.activation(t_co[:3 * cap, :], pt[:, :], ACT.Prelu,
                                     alpha=t_alpha[:3 * cap, :1])
                # transpose -> [128, 3cap], then pool along free dim
                ptt = ps.tile([128, 3 * 42], BF16, tag="pst")
                nc.tensor.transpose(ptt[:, :3 * cap], t_co[:3 * cap, :],
                                    t_id[:3 * cap, :3 * cap])
                t_coT = sb.tile([128, 3 * 42], BF16, tag="coT")
                nc.vector.tensor_copy(t_coT[:, :3 * cap], ptt[:, :3 * cap])
                # pooled sum over the 3 windows (1/3 folded into wfc1 host-side)
                t_pool = sb.tile([128, 42], BF16, tag="pool")
                nc.vector.tensor_tensor(t_pool[:, :cap], t_coT[:, :cap],
                                        t_coT[:, cap:2 * cap], op=ALU.add)
                nc.vector.tensor_tensor(t_pool[:, :cap], t_pool[:, :cap],
                                        t_coT[:, 2 * cap:3 * cap], op=ALU.add)

                # fc1: psum [cap, 128]
                pt1 = ps.tile([42, 128], F32, tag="psfc1")
                nc.tensor.matmul(pt1[:cap, :], t_pool[:, :cap], t_w1a[:, :],
                                 start=True, stop=False)
                nc.tensor.matmul(pt1[:cap, :], t_char[:, pos0[s]:pos0[s] + cap],
                                 t_w1b[:32, :], start=False, stop=False)
                nc.tensor.matmul(pt1[:cap, :], t_ones[:1, :cap],
                                 t_w1c[:1, :], start=False, stop=True)
                t_h1s = sb.tile([42, 128], F32, tag="h1s")
                nc.scalar.activation(t_h1s[:cap, :], pt1[:cap, :], ACT.Relu)
                # fc2 (slot-local, DVE)
                t_w2s = sb.tile([42, 128], F32, tag="w2s")
                nc.sync.dma_start(t_w2s[:, :], w2rep[s, :, :])
                t_b2s = sb.tile([42, 1], F32, tag="b2s")
                nc.sync.dma_start(t_b2s[:, :], b2rep[s, :, :])
                t_prod = sb.tile([42, 128], F32, tag="prod")
                nc.vector.tensor_tensor(t_prod[:cap, :], t_h1s[:cap, :],
                                        t_w2s[:cap, :], op=ALU.mult)
                t_pcs = sb.tile([42, 1], F32, tag="pcs")
                nc.vector.tensor_reduce(t_pcs[:cap, :], t_prod[:cap, :],
                                        op=ALU.add, axis=mybir.AxisListType.X)
                nc.vector.tensor_tensor(t_pcs[:cap, :], t_pcs[:cap, :],
                                        t_b2s[:cap, :], op=ALU.add)
                nc.sync.dma_start(pchar_out[pos0[s]:pos0[s] + cap, :],
                                  t_pcs[:cap, :])

    return nc


# ---------------------------------------------------------------------------
# Host-side prep
# ---------------------------------------------------------------------------

def _edge_mask_win(lv, q):
    w_, hh, (mm, oo), ext, win = GEOM[lv]
    lo = mm * q + oo
    m = np.ones(w_, np.float32)
    js = np.arange(w_)
    m[(lo + js < 0) | (lo + js >= ext)] = 0.0
    return np.concatenate([m[:win], m[w_ - win:]])


def host_prep_backbone(inp):
    """Returns list of 8 input dicts."""
    x = np.asarray(inp['x'])
    g_style = np.asarray(inp['g_style'])
    sp1_w, sp2_w = np.asarray(inp['sp1_w']), np.asarray(inp['sp2_w'])
    in_w, in_b = np.asarray(inp['in_w']), np.asarray(inp['in_b'])
    c1w1, c1b1 = np.asarray(inp['c1w1']), np.asarray(inp['c1b1'])
    c1w2, c1b2 = np.asarray(inp['c1w2']), np.asarray(inp['c1b2'])
    c2w, c2b = np.asarray(inp['c2w']), np.asarray(inp['c2b'])
    c3w1, c3b1 = np.asarray(inp['c3w1']), np.asarray(inp['c3b1'])
    c3w2, c3b2 = np.asarray(inp['c3w2']), np.asarray(inp['c3b2'])
    fm_w, fm_b = np.asarray(inp['fm_w']), np.asarray(inp['fm_b'])
    lab = np.asarray(inp['label'])
    sc = np.asarray(inp['spaced_style'])

    # shared (core-independent) weight tensors
    w1 = np.ascontiguousarray(in_w[:, 0].reshape(64, 49).T).astype(BF)  # [49, 64]
    b1 = in_b[:, None]
    w2 = np.concatenate([c1w1[:, :64, dh, dw].T for dh in range(3) for dw in range(3)], axis=1)
    w3 = np.concatenate([0.25 * c1w2[:, :, dh, dw].T for dh in range(3) for dw in range(3)], axis=1)
    b3 = c1b2[:, None]
    w4 = np.concatenate([c2w[:, :128, dh, dw].T for dh in range(3) for dw in range(3)], axis=1)
    w5 = np.concatenate([0.25 * c3w1[:, :128, dh, dw].T for dh in range(3) for dw in range(3)], axis=1)
    b5 = c3b1[:, None]
    wsal = np.concatenate([c3w1[:, 128:, :, dw].sum(axis=2).T for dw in range(3)], axis=1)  # [112, 3*128]
    w6 = np.concatenate([0.25 * c3w2[cb * 128:(cb + 1) * 128, :, dh, dw].T
                         for dh in range(3) for dw in range(3)
                         for cb in range(2)], axis=1)                 # [128, 18*128]
    b6 = np.stack([c3b2[:128], c3b2[128:]], axis=1)                   # [128, 2]
    wfm = np.stack([fm_w[0, cb * 128:(cb + 1) * 128, dh, dw]
                    for cb in range(2)
                    for dh in range(3) for dw in range(3)], axis=1)   # [128, 18]
    bfm = fm_b[:, None]

    w_sty1 = c1w1[:, 64:]
    w_sty2 = c2w[:, 128:]

    ins = []
    for c in range(N_CORES):
        b, q = divmod(c, 4)
        style1 = sp1_w @ g_style[b]
        style2 = sp2_w @ g_style[b]
        bias1 = c1b1 + np.einsum('ocij,c->o', w_sty1, style1)
        bias2 = c2b + np.einsum('ocij,c->o', w_sty2, style2)

        lo0 = 128 * q - 36
        xs = np.zeros((64, 200), np.float32)
        s_, e_ = max(0, lo0), min(512, lo0 + 200)
        xs[:, s_ - lo0:e_ - lo0] = x[b, 0][:, s_:e_]
        # im2col: x49[(dh*7+dw), (h0*194 + w')] = xs[h0+dh, w'+dw]
        x49 = np.zeros((49, 58 * 194), np.float32)
        for dh in range(7):
            for dw in range(7):
                x49[dh * 7 + dw] = xs[dh:dh + 58, dw:dw + 194].reshape(-1)

        cw2_ = np.zeros((64, 192), np.float32)
        lo2 = 128 * q - 32
        if 0 <= 0 - lo2 < 192:
            cw2_[:, 0 - lo2] = -np.einsum('oci,c->o', w_sty1[:, :, :, 0], style1)
        if 0 <= 511 - lo2 < 192:
            cw2_[:, 511 - lo2] = -np.einsum('oci,c->o', w_sty1[:, :, :, 2], style1)
        cw4_ = np.zeros((128, 92), np.float32)
        lo4 = 64 * q - 14
        if 0 <= 0 - lo4 < 92:
            cw4_[:, 0 - lo4] = -np.einsum('oci,c->o', w_sty2[:, :, :, 0], style2)
        if 0 <= 255 - lo4 < 92:
            cw4_[:, 255 - lo4] = -np.einsum('oci,c->o', w_sty2[:, :, :, 2], style2)

        sal_full = np.concatenate([sc[:, b, :], lab[:, b, :]], axis=1).T  # [112,128]
        lo5 = 32 * q - 7
        sal_ = np.zeros((112, 46), np.float32)
        s_, e_ = max(0, lo5), min(128, lo5 + 46)
        sal_[:, s_ - lo5:e_ - lo5] = sal_full[:, s_:e_]

        d = dict(x49=x49.astype(BF), w1=w1, b1=b1, w2=w2, b2=bias1[:, None], cw2=cw2_,
                 w3=w3, b3=b3, w4=w4, b4=bias2[:, None], cw4=cw4_,
                 w5=w5, b5=b5, wsal=wsal, sal=sal_, w6=w6, b6=b6,
                 wfm=wfm, bfm=bfm)
        for lv in ('h', 'g', 'm', 'g2', 'g3', 'g4'):
            p = 64 if lv in ('h', 'g') else 128
            mw = _edge_mask_win(lv, q)
            d[f"mask_{lv}"] = np.broadcast_to(mw, (p, mw.size)).copy()
        ins.append({k: np.ascontiguousarray(v) if v.dtype == BF
                    else np.ascontiguousarray(v, np.float32) for k, v in d.items()})
    return ins


def host_route_experts(inp, mL_full):
    """Build expert-phase inputs.  Returns (caps, ins_list, tokmaps)."""
    lab = np.asarray(inp['label'])
    cls = lab.argmax(axis=2).T  # [B, T]
    B, T = cls.shape
    cd_conv_w = np.asarray(inp['cd_conv_w'])
    cd_conv_b = np.asarray(inp['cd_conv_b'])
    cd_fc1_w = np.asarray(inp['cd_fc1_w'])
    cd_fc1_b = np.asarray(inp['cd_fc1_b'])
    cd_fc2_w = np.asarray(inp['cd_fc2_w'])
    cd_fc2_b = np.asarray(inp['cd_fc2_b'])
    char_style = np.asarray(inp['char_style'])

    # expert -> token list
    etok = {}
    for b in range(B):
        for t in range(T):
            etok.setdefault(int(cls[b, t]), []).append((b, t))
    experts = sorted(etok, key=lambda e: -len(etok[e]))
    # greedy bin-pack experts onto cores by token count
    S = int(np.ceil(len(experts) / N_CORES))
    core_exp = [[] for _ in range(N_CORES)]
    core_load = np.zeros(N_CORES, int)
    for e in experts:
        cands = sorted(range(N_CORES), key=lambda c: (core_load[c],))
        for c in cands:
            if len(core_exp[c]) < S:
                core_exp[c].append(e)
                core_load[c] += len(etok[e])
                break
    # capacity profile: per slot rank, max count across cores
    caps = np.zeros(S, int)
    for c in range(N_CORES):
        core_exp[c].sort(key=lambda e: -len(etok[e]))
        for i, e in enumerate(core_exp[c]):
            caps[i] = max(caps[i], len(etok[e]))
    caps = np.maximum(caps, 1)
    NTOT = int(caps.sum())
    pos0 = np.concatenate([[0], np.cumsum(caps)]).astype(int)
    NROWS = NTOT * 5

    mLp = np.pad(mL_full, ((0, 0), (0, 0), (0, 0), (2, 2)))  # [B,256,3,68]

    ins = []
    tokmaps = []
    for c in range(N_CORES):
        patches = np.zeros((128, 18, 3 * NTOT), np.float32)
        wexp = np.zeros((S, 128, 19, 128), np.float32)
        wfc1a = np.zeros((S, 128, 128), np.float32)
        wfc1b = np.zeros((S, 33, 128), np.float32)
        charT = np.zeros((32, NTOT), np.float32)
        w2rep = np.zeros((S, 42, 128), np.float32)
        b2rep = np.zeros((S, 42, 1), np.float32)
        tokmap = []  # (canonical_pos, b, t)
        for si, e in enumerate(core_exp[c]):
            toks = etok[e]
            We = cd_conv_w[e]  # [128, 256, 3, 3]
            # wexp[k=(r*3+s3)*2+cb] = We[:, cb*128:(cb+1)*128, r, s3].T
            for rr in range(3):
                for s3 in range(3):
                    for cb in range(2):
                        k = (rr * 3 + s3) * 2 + cb
                        wexp[si, :, k, :] = We[:, cb * 128:(cb + 1) * 128, rr, s3].T
            wexp[si, 0, 18, :] = cd_conv_b[e]
            wfc1a[si] = cd_fc1_w[e][:128] / 3.0
            wfc1b[si, :32] = cd_fc1_w[e][128:]
            wfc1b[si, 32] = cd_fc1_b[e]
            for j, (b, t) in enumerate(toks):
                pos = pos0[si] + j
                idx = t // 2
                blk = mLp[b, :, :, idx:idx + 5]  # [256, 3, 5]
                cap_s = int(caps[si])
                # chunk k=(r*3+s3)*2+cb; slot cols ordered (w outer, t inner)
                # to match the device psum M-order and window pooling
                for rr in range(3):
                    for s3 in range(3):
                        for cb in range(2):
                            k = (rr * 3 + s3) * 2 + cb
                            seg = blk[cb * 128:(cb + 1) * 128, rr, s3:s3 + 3]
                            for w in range(3):
                                patches[:, k, 3 * pos0[si] + w * cap_s + j] = seg[:, w]
                charT[:, pos] = char_style[b, e]
                w2rep[si, j] = cd_fc2_w[e][:, 0]
                b2rep[si, j, 0] = cd_fc2_b[e][0]
                tokmap.append((pos, b, t))
        ins.append(dict(
            patches=np.ascontiguousarray(
                patches.reshape(128, -1).astype(BF)),
            wexp=np.ascontiguousarray(wexp.reshape(S, 128, 19 * 128).astype(BF)),
            wfc1a=np.ascontiguousarray(wfc1a.astype(BF)),
            wfc1b=np.ascontiguousarray(wfc1b.astype(BF)),
            charT=np.ascontiguousarray(charT.astype(BF)),
            w2rep=w2rep, b2rep=b2rep))
        tokmaps.append(tokmap)
    return caps, ins, tokmaps


# ---------------------------------------------------------------------------
# Entry point
# ---------------------------------------------------------------------------

def kernel(**inputs):
    inp = {k: np.asarray(v) for k, v in inputs.items()}
    B, T = 2, 128

    nc1 = build_backbone()
    ins1 = host_prep_backbone(inp)
    res1 = run_bass_kernel_spmd(nc1, ins1, list(range(N_CORES))).results

    mL_full = np.zeros((B, 256, 3, 64), np.float32)
    pM = np.zeros((B, 1, 1, 64), np.float32)
    for c in range(N_CORES):
        b, q = divmod(c, 4)
        mo = res1[c]["mL_out"]  # [2, 128, 3, 16]
        mL_full[b, :128, :, 16 * q:16 * q + 16] = mo[0]
        mL_full[b, 128:, :, 16 * q:16 * q + 16] = mo[1]
        pM[b, 0, 0, 16 * q:16 * q + 16] = res1[c]["pM_out"][0]

    caps, ins2, tokmaps = host_route_experts(inp, mL_full)
    nc2 = build_expert(caps)
    res2 = run_bass_kernel_spmd(nc2, ins2, list(range(N_CORES))).results

    pChar = np.zeros((B * T, 1), np.float32)
    for c in range(N_CORES):
        pc = res2[c]["pchar_out"]
        for (pos, b, t) in tokmaps[c]:
            pChar[b * T + t, 0] = pc[pos, 0]

    return pM, pChar


# revision 17
# speedup vs baseline: 1.0058x; 1.0058x over previous
"""Trainium2 Bass kernel for nn_CharCondDiscriminatorAP (moe_routing).

Strategy (8 NeuronCores, SPMD):
- Launch 1 (backbone): data-parallel over (batch=2) x (W quarters=4).
  Each core runs the full conv stack on a W-slab with halos.  All per-core
  differences (slab data, folded style biases, edge masks, style edge
  corrections, sal slabs) are host-staged inputs so one program serves all
  cores.  Style contributions are folded into per-channel biases; pooling is
  implemented as 2x2 sums with the 1/4 folded into the next conv's weights.
  Outputs: per-core mL slice [256,3,16] and pM slice [1,16].
- Host routing: tokens grouped by class (expert), experts bin-packed onto
  cores, patch windows gathered from mL into a canonical matmul-ready layout.
- Launch 2 (experts): each core holds ~1/8 of the used CharDisc weight banks
  (bf16) and computes conv+fc for its tokens' patches.
"""
import sys
sys.path.insert(0, '/opt/trn_rl_repo')
sys.path.insert(0, '/root/.axon_site')
import numpy as np
import ml_dtypes

import bass_rust
import concourse.bass as bass
import concourse.mybir as mybir
from concourse.bass import AP
from concourse.bass_utils import run_bass_kernel_spmd
from concourse.tile import TileContext
from concourse.masks import make_identity

F32 = mybir.dt.float32
F32R = mybir.dt.float32r
BF16 = mybir.dt.bfloat16
ALU = mybir.AluOpType
ACT = mybir.ActivationFunctionType
LEAK = 0.1
BF = ml_dtypes.bfloat16

N_CORES = 8

# backbone slab geometry: level -> (width, h_rows, lo(q), true_extent, mask_win)
# lo(q) given as (mult, off): lo = mult*q + off
GEOM = {
    'h':  (194, 58, (128, -33), 512, 33),
    'g':  (192, 56, (128, -32), 512, 32),
    'hp': (96,  28, (64, -16), 256, 0),
    'm':  (94,  26, (64, -15), 256, 15),
    'g2': (92,  24, (64, -14), 256, 14),
    'mL': (46,  12, (32, -7), 128, 0),
    'g3': (44,  10, (32, -6), 128, 6),
    'mL2': (22, 5,  (16, -3), 64, 0),
    'g4': (20,  3,  (16, -2), 64, 2),
}


class PatchedTileContext(TileContext):
    """This container's walrus allows only ONE sync-wait per instruction on
    several opcodes; stock Tile attaches one wait per dependency processor.
    Post-pass: hoist excess waits onto injected same-engine NoOps placed
    immediately before the instruction (sequencers execute in order, so
    waiting on the nops first is equivalent)."""

    MAX_WAITS = 1

    def __exit__(self, *args):
        res = super().__exit__(*args)
        self._split_excess_waits()
        return res

    def _split_excess_waits(self):
        nc = self.nc
        nop_id = [0]

        def fix_block(bb):
            insts = list(bb.instructions)
            out = []
            changed = False
            for inst in insts:
                si = inst.sync_info
                if si is not None and len(si.on_wait) > self.MAX_WAITS:
                    waits = list(si.on_wait)
                    keep = waits[-self.MAX_WAITS:]
                    hoist = waits[:-self.MAX_WAITS]
                    for w in hoist:
                        nop = mybir.InstNoOp(
                            name=f"I-waitfix-{nop_id[0]}", ins=[], outs=[])
                        nop_id[0] += 1
                        nop.engine = inst.engine
                        nop.sync_info = bass_rust.SyncInfo(
                            on_wait=[w], on_update=[])
                        out.append(nop)
                    inst.sync_info = bass_rust.SyncInfo(
                        on_wait=keep, on_update=list(si.on_update))
                    changed = True
                out.append(inst)
            if changed:
                try:
                    bb.instructions = out
                except Exception:
                    bb.instructions.clear()
                    bb.instructions.extend(out)

        for fn in nc.m.functions:
            for bb in fn.blocks:
                fix_block(bb)

    def _drain_and_barrier(self, tick_clock, wait_clock):
        nc = self.nc
        gc = tick_clock.global_clock
        nz = [(p, t) for p, t in enumerate(list(gc)) if t > 0]
        for p, t in nz[:-1]:
            v = bass_rust.VectorClock()
            v.require_at_least(p, t)
            nop = nc.sync.nop()
            wait_clock.add_sem_waits(nop.ins, bass_rust.ScopedClock({None: v}))
        drain_inst = nc.sync.drain()
        if nz:
            p, t = nz[-1]
            v = bass_rust.VectorClock()
            v.require_at_least(p, t)
            wait_clock.add_sem_waits(drain_inst.ins, bass_rust.ScopedClock({None: v}))
        nc.all_engine_barrier()
        assert self.sems is not None
        popped = nc._tile_sem_poison_stack.pop()
        assert popped is self._sem_poison
        nc.clear_and_free_semaphores(list(self.sems.allocated().values()))
        nc.all_engine_barrier()


def _bcast_h(ap2d, h):
    """Insert a step-0 (broadcast) dim of count h between partition and free."""
    dims = [list(d) for d in ap2d.ap]
    assert len(dims) == 2
    return AP(ap2d.tensor, ap2d.offset, [dims[0], [0, h], dims[1]])


# ---------------------------------------------------------------------------
# Backbone kernel builder
# ---------------------------------------------------------------------------

def build_backbone():
    nc = bass.Bass()
    dram = {}

    def din(name, shape, dt=F32):
        dram[name] = nc.dram_tensor(name, shape, dt, kind="ExternalInput")
        return dram[name]

    x49 = din("x49", [49, 58 * 194], BF16)
    w1 = din("w1", [49, 64], BF16)
    b1 = din("b1", [64, 1])
    w2 = din("w2", [64, 9 * 64])
    b2 = din("b2", [64, 1])
    cw2 = din("cw2", [64, 192])      # corr for L2 (full width, mostly zero)
    w3 = din("w3", [64, 9 * 128])
    b3 = din("b3", [128, 1])
    w4 = din("w4", [128, 9 * 128])
    b4 = din("b4", [128, 1])
    cw4 = din("cw4", [128, 92])
    w5 = din("w5", [128, 9 * 128])
    b5 = din("b5", [128, 1])
    wsal = din("wsal", [112, 3 * 128])
    sal = din("sal", [112, 46])
    w6 = din("w6", [128, 18 * 128])
    b6 = din("b6", [128, 2])
    wfm = din("wfm", [128, 18])
    bfm = din("bfm", [1, 1])
    # per-level edge masks [P, 2*win] (left win cols | right win cols)
    mk = {}
    for lv in ('h', 'g', 'm', 'g2', 'g3', 'g4'):
        w_, hh, _, _, win = GEOM[lv]
        p = 64 if lv in ('h', 'g') else 128
        mk[lv] = din(f"mask_{lv}", [p, 2 * win])

    mL_out = nc.dram_tensor("mL_out", [2, 128, 3, 16], F32, kind="ExternalOutput")
    pM_out = nc.dram_tensor("pM_out", [1, 16], F32, kind="ExternalOutput")

    def r(ap):
        return ap if ap.dtype == F32R else ap.bitcast(F32R)

    with PatchedTileContext(nc) as tc:
        with tc.tile_pool(name="wt", bufs=1) as wt, \
             tc.tile_pool(name="act", bufs=1) as actp, \
             tc.tile_pool(name="ps", bufs=8, space="PSUM") as ps:

            # ---- load weights/support tensors
            t_x49 = actp.tile([49, 58 * 194], BF16, tag="bigA")
            nc.sync.dma_start(t_x49[:, :], x49[:, :])
            t_w1 = wt.tile([49, 64], BF16)
            nc.sync.dma_start(t_w1[:, :], w1[:, :])
            t_b1 = wt.tile([64, 1], F32)
            nc.sync.dma_start(t_b1[:, :], b1[:, :])
            def round_load(dst, dram_ap):
                stg = wt.tile(list(dst.shape), F32, name="wstage", tag="wstage")
                nc.sync.dma_start(stg[:, :], dram_ap)
                nc.vector.tensor_copy(dst[:, :], stg[:, :])

            t_w2 = wt.tile([64, 9 * 64], F32R)
            round_load(t_w2, w2[:, :])
            t_b2 = wt.tile([64, 1], F32)
            nc.sync.dma_start(t_b2[:, :], b2[:, :])
            t_cw2 = wt.tile([64, 192], F32)
            nc.sync.dma_start(t_cw2[:, :], cw2[:, :])
            t_w3 = wt.tile([64, 9 * 128], F32R)
            round_load(t_w3, w3[:, :])
            t_b3 = wt.tile([128, 1], F32)
            nc.sync.dma_start(t_b3[:, :], b3[:, :])
            t_w4 = wt.tile([128, 9 * 128], F32R)
            round_load(t_w4, w4[:, :])
            t_b4 = wt.tile([128, 1], F32)
            nc.sync.dma_start(t_b4[:, :], b4[:, :])
            t_cw4 = wt.tile([128, 92], F32)
            nc.sync.dma_start(t_cw4[:, :], cw4[:, :])
            t_w5 = wt.tile([128, 9 * 128], F32R)
            round_load(t_w5, w5[:, :])
            t_b5 = wt.tile([128, 1], F32)
            nc.sync.dma_start(t_b5[:, :], b5[:, :])
            t_wsal = wt.tile([112, 3 * 128], F32R)
            round_load(t_wsal, wsal[:, :])
            t_sal = wt.tile([112, 46], F32)
            nc.sync.dma_start(t_sal[:, :], sal[:, :])
            t_w6 = wt.tile([128, 18 * 128], F32R)
            round_load(t_w6, w6[:, :])
            t_b6 = wt.tile([128, 2], F32)
            nc.sync.dma_start(t_b6[:, :], b6[:, :])
            t_wfm = wt.tile([128, 18], F32R)
            round_load(t_wfm, wfm[:, :])
            t_bfm = wt.tile([1, 1], F32)
            nc.sync.dma_start(t_bfm[:, :], bfm[:, :])
            t_alpha = wt.tile([128, 1], F32)
            nc.vector.memset(t_alpha[:, :], LEAK)
            t_mk = {}
            for lv in ('h', 'g', 'm', 'g2', 'g3', 'g4'):
                _, _, _, _, win = GEOM[lv]
                p = 64 if lv in ('h', 'g') else 128
                t_mk[lv] = wt.tile([p, 2 * win], F32, name=f"mask{lv}",
                                   tag=f"mask{lv}")
                nc.sync.dma_start(t_mk[lv][:, :], mk[lv][:, :])

            def apply_mask(slab, lv, P):
                w_, hh, _, _, win = GEOM[lv]
                s3 = slab[:, :].rearrange("p (h w) -> p h w", h=hh)
                # left window
                nc.vector.tensor_tensor(
                    s3[:, :, :win], s3[:, :, :win],
                    _bcast_h(t_mk[lv][:P, :win], hh), op=ALU.mult)
                nc.vector.tensor_tensor(
                    s3[:, :, w_ - win:], s3[:, :, w_ - win:],
                    _bcast_h(t_mk[lv][:P, win:], hh), op=ALU.mult)

            # ---- L1: conv1 7x7 (K=49) -> h [64, 58*194]
            t_h = actp.tile([64, 58 * 194], F32R, tag="bigB")
            for i in range(29):
                off = 2 * i * 194
                pt = ps.tile([64, 388], F32, tag="pp")
                nc.tensor.matmul(pt[:, :], t_w1[:, :], t_x49[:, off:off + 388],
                                 start=True, stop=True)
                nc.scalar.activation(t_h[:, off:off + 388], pt[:, :], ACT.Prelu,
                                     bias=t_b1[:, :1], alpha=t_alpha[:64, :1])
            apply_mask(t_h, 'h', 64)

            # ---- L2: c1w1 3x3 (K=64, folded style) -> g [64, 56*192]
            t_g = actp.tile([64, 56 * 192], F32R, tag="bigC")
            for i in range(28):
                rows = 2
                pt = ps.tile([64, 384], F32, tag="pp")
                for k in range(9):
                    dh, dw = divmod(k, 3)
                    rhs = AP(t_h.tensor, t_h[:, :].offset + (2 * i + dh) * 194 + dw,
                             [list(t_h[:, :].ap[0]), [194, rows], [1, 192]])
                    nc.tensor.matmul(pt[:, :], r(t_w2[:, k * 64:(k + 1) * 64]), r(rhs),
                                     start=(k == 0), stop=(k == 8))
                # style edge correction (pre-lrelu), broadcast over the 2 rows
                pt3 = pt[:, :].rearrange("p (h w) -> p h w", h=rows)
                nc.vector.tensor_tensor(pt3[:, :, :], pt3[:, :, :],
                                        _bcast_h(t_cw2[:, :], rows), op=ALU.add)
                nc.scalar.activation(t_g[:, i * 2 * 192:(i + 1) * 2 * 192], pt[:, :],
                                     ACT.Prelu, bias=t_b2[:, :1], alpha=t_alpha[:64, :1])
            apply_mask(t_g, 'g', 64)

            # ---- pool g -> hp [64, 28*96] (sum; /4 folded into w3)
            t_gt = actp.tile([64, 56 * 96], F32R, tag="bigA")  # w-paired
            g3d = t_g[:, :].rearrange("p (a two) -> p a two", two=2)
            nc.vector.tensor_tensor(t_gt[:, :], g3d[:, :, 0], g3d[:, :, 1], op=ALU.add)
            t_hp = actp.tile([64, 28 * 96], F32R, tag="bigD")
            gt3 = t_gt[:, :].rearrange("p (h two w) -> p h two w", two=2, w=96)
            nc.vector.tensor_tensor(t_hp[:, :].rearrange("p (h w) -> p h w", w=96),
                                    gt3[:, :, 0, :], gt3[:, :, 1, :], op=ALU.add)

            # ---- L3: c1w2 (K=64) -> m [128, 26*94]
            t_m = actp.tile([128, 26 * 94], F32R, tag="bigB")
            r0 = 0
            for i in range(7):
                rows = min(4, 26 - r0)
                pt = ps.tile([128, 4 * 94], F32, tag="pp")
                for k in range(9):
                    dh, dw = divmod(k, 3)
                    rhs = AP(t_hp.tensor, t_hp[:, :].offset + (r0 + dh) * 96 + dw,
                             [list(t_hp[:, :].ap[0]), [96, rows], [1, 94]])
                    nc.tensor.matmul(pt[:, :rows * 94], r(t_w3[:, k * 128:(k + 1) * 128]), r(rhs),
                                     start=(k == 0), stop=(k == 8))
                nc.scalar.activation(t_m[:, r0 * 94:(r0 + rows) * 94],
                                     pt[:, :rows * 94],
                                     ACT.Prelu, bias=t_b3[:, :1], alpha=t_alpha[:, :1])
                r0 += rows
            apply_mask(t_m, 'm', 128)

            # ---- L4: c2w (K=128, folded style2) -> g2 [128, 24*92]
            t_g2 = actp.tile([128, 24 * 92], F32R, tag="bigC")
            for i in range(6):
                rows = 4
                pt = ps.tile([128, 4 * 92], F32, tag="pp")
                for k in range(9):
                    dh, dw = divmod(k, 3)
                    rhs = AP(t_m.tensor, t_m[:, :].offset + (4 * i + dh) * 94 + dw,
                             [list(t_m[:, :].ap[0]), [94, rows], [1, 92]])
                    nc.tensor.matmul(pt[:, :], r(t_w4[:, k * 128:(k + 1) * 128]), r(rhs),
                                     start=(k == 0), stop=(k == 8))
                pt3 = pt[:, :].rearrange("p (h w) -> p h w", h=rows)
                nc.vector.tensor_tensor(pt3[:, :, :], pt3[:, :, :],
                                        _bcast_h(t_cw4[:, :], rows), op=ALU.add)
                nc.scalar.activation(t_g2[:, i * 4 * 92:(i + 1) * 4 * 92], pt[:, :],
                                     ACT.Prelu, bias=t_b4[:, :1], alpha=t_alpha[:, :1])
            apply_mask(t_g2, 'g2', 128)

            # ---- pool g2 -> mL [128, 12*46]
            t_g2t = actp.tile([128, 24 * 46], F32R, tag="bigA")
            g23 = t_g2[:, :].rearrange("p (a two) -> p a two", two=2)
            nc.vector.tensor_tensor(t_g2t[:, :], g23[:, :, 0], g23[:, :, 1], op=ALU.add)
            t_mL = actp.tile([128, 12 * 46], F32R)
            g2t3 = t_g2t[:, :].rearrange("p (h two w) -> p h two w", two=2, w=46)
            nc.vector.tensor_tensor(t_mL[:, :].rearrange("p (h w) -> p h w", w=46),
                                    g2t3[:, :, 0, :], g2t3[:, :, 1, :], op=ALU.add)

            # ---- L5: c3w1 (K=128 main + K=112 sal) -> g3 [128, 10*44]
            t_salb = actp.tile([112, 10 * 46], F32R)
            nc.vector.tensor_copy(
                t_salb[:, :].rearrange("p (h w) -> p h w", w=46),
                _bcast_h(t_sal[:, :], 10))
            t_g3 = actp.tile([128, 10 * 44], F32R)
            pt = ps.tile([128, 440], F32, tag="pp")
            for k in range(9):
                dh, dw = divmod(k, 3)
                rhs = AP(t_mL.tensor, t_mL[:, :].offset + dh * 46 + dw,
                         [list(t_mL[:, :].ap[0]), [46, 10], [1, 44]])
                nc.tensor.matmul(pt[:, :], r(t_w5[:, k * 128:(k + 1) * 128]), r(rhs),
                                 start=(k == 0), stop=False)
            for dw in range(3):
                rhs = AP(t_salb.tensor, t_salb[:, :].offset + dw,
                         [list(t_salb[:, :].ap[0]), [46, 10], [1, 44]])
                nc.tensor.matmul(pt[:, :], r(t_wsal[:, dw * 128:(dw + 1) * 128]), r(rhs),
                                 start=False, stop=(dw == 2))
            nc.scalar.activation(t_g3[:, :], pt[:, :], ACT.Prelu,
                                 bias=t_b5[:, :1], alpha=t_alpha[:, :1])
            apply_mask(t_g3, 'g3', 128)

            # ---- pool g3 -> mL2 [128, 5*22]
            t_g3t = actp.tile([128, 10 * 22], F32R, tag="bigD")
            g33 = t_g3[:, :].rearrange("p (a two) -> p a two", two=2)
            nc.vector.tensor_tensor(t_g3t[:, :], g33[:, :, 0], g33[:, :, 1], op=ALU.add)
            t_mL2 = actp.tile([128, 5 * 22], F32R, tag="bigB")
            g3t3 = t_g3t[:, :].rearrange("p (h two w) -> p h two w", two=2, w=22)
            nc.vector.tensor_tensor(t_mL2[:, :].rearrange("p (h w) -> p h w", w=22),
                                    g3t3[:, :, 0, :], g3t3[:, :, 1, :], op=ALU.add)

            # ---- L6: c3w2 (K=128, Cout=256 in 2 chunks) -> g4 [128, 2*3*20]
            t_g4 = actp.tile([128, 2 * 3 * 20], F32R)
            for cb in range(2):
                pt = ps.tile([128, 60], F32, tag="pp")
                for k in range(9):
                    dh, dw = divmod(k, 3)
                    rhs = AP(t_mL2.tensor, t_mL2[:, :].offset + dh * 22 + dw,
                             [list(t_mL2[:, :].ap[0]), [22, 3], [1, 20]])
                    nc.tensor.matmul(pt[:, :], r(t_w6[:, (k * 2 + cb) * 128:(k * 2 + cb) * 128 + 128]), r(rhs),
                                     start=(k == 0), stop=(k == 8))
                nc.scalar.activation(t_g4[:, cb * 60:(cb + 1) * 60], pt[:, :],
                                     ACT.Prelu, bias=t_b6[:, cb:cb + 1], alpha=t_alpha[:, :1])
            # mask g4: left/right 2 cols of each (cb, r) row-block
            win = 2
            g44 = t_g4[:, :].rearrange("p (cr w) -> p cr w", w=20)
            mml = AP(t_mk['g4'].tensor, t_mk['g4'][:, :].offset,
                     [list(t_mk['g4'][:, :].ap[0]), [0, 6], [1, win]])
            mmr = AP(t_mk['g4'].tensor, t_mk['g4'][:, :].offset + win,
                     [list(t_mk['g4'][:, :].ap[0]), [0, 6], [1, win]])
            nc.vector.tensor_tensor(g44[:, :, :win], g44[:, :, :win], mml, op=ALU.mult)
            nc.vector.tensor_tensor(g44[:, :, 20 - win:], g44[:, :, 20 - win:],
                                    mmr, op=ALU.mult)

            # ---- fm conv -> pM [1, 16]
            ptf = ps.tile([1, 16], F32, tag="pp")
            kk = 0
            for cb in range(2):
                for k in range(9):
                    dh, dw = divmod(k, 3)
                    rhs = t_g4[:, cb * 60 + dh * 20 + 1 + dw:
                               cb * 60 + dh * 20 + 1 + dw + 16]
                    nc.tensor.matmul(ptf[:, :], r(t_wfm[:, cb * 9 + k:cb * 9 + k + 1]), r(rhs),
                                     start=(kk == 0), stop=(kk == 17))
                    kk += 1
            t_pM = actp.tile([1, 16], F32)
            nc.scalar.activation(t_pM[:, :], ptf[:, :], ACT.Identity,
                                 bias=t_bfm[:1, :1])
            nc.sync.dma_start(pM_out[:, :], t_pM[:, :])

            # ---- mL slice out: g4[:, cb, :, 2:18] -> mL_out[cb]
            for cb in range(2):
                src = AP(t_g4.tensor, t_g4[:, :].offset + cb * 60 + 2,
                         [list(t_g4[:, :].ap[0]), [20, 3], [1, 16]]).bitcast(F32)
                nc.sync.dma_start(mL_out[cb, :, :, :], src)

    return nc


# ---------------------------------------------------------------------------
# Expert kernel builder (parameterized by slot capacity profile)
# ---------------------------------------------------------------------------

def build_expert(caps):
    S = len(caps)
    NTOT = int(sum(caps))
    NROWS = NTOT * 5
    pos0 = [int(v) for v in np.concatenate([[0], np.cumsum(caps)])]

    nc = bass.Bass()
    patches = nc.dram_tensor("patches", [128, 18 * 3 * NTOT], BF16,
                             kind="ExternalInput")
    wexp = nc.dram_tensor("wexp", [S, 128, 19 * 128], BF16, kind="ExternalInput")
    wfc1a = nc.dram_tensor("wfc1a", [S, 128, 128], BF16, kind="ExternalInput")
    wfc1b = nc.dram_tensor("wfc1b", [S, 33, 128], BF16, kind="ExternalInput")
    charT = nc.dram_tensor("charT", [32, NTOT], BF16, kind="ExternalInput")
    w2rep = nc.dram_tensor("w2rep", [S, 42, 128], F32, kind="ExternalInput")
    b2rep = nc.dram_tensor("b2rep", [S, 42, 1], F32, kind="ExternalInput")
    pchar_out = nc.dram_tensor("pchar_out", [NTOT, 1], F32, kind="ExternalOutput")

    with PatchedTileContext(nc) as tc:
        with tc.tile_pool(name="sb", bufs=1) as sb, \
             tc.tile_pool(name="wp", bufs=3) as wp, \
             tc.tile_pool(name="ps", bufs=2, space="PSUM") as ps:

            t_p = sb.tile([128, 18 * 3 * NTOT], BF16)
            nc.sync.dma_start(t_p[:, :], patches[:, :])
            t_char = sb.tile([32, NTOT], BF16)
            nc.sync.dma_start(t_char[:, :], charT[:, :])
            t_ones = sb.tile([1, 128], BF16)
            nc.vector.memset(t_ones[:, :], 1.0)
            t_alpha = sb.tile([128, 1], F32)
            nc.vector.memset(t_alpha[:, :], LEAK)
            t_id = sb.tile([128, 128], BF16)
            make_identity(nc, t_id[:, :])

            for s in range(S):
                cap = int(caps[s])
                t_w = wp.tile([128, 19 * 128], BF16, tag="wexp")
                nc.sync.dma_start(t_w[:, :], wexp[s, :, :])
                t_w1a = wp.tile([128, 128], BF16, tag="wfc1a")
                nc.sync.dma_start(t_w1a[:, :], wfc1a[s, :, :])
                t_w1b = wp.tile([32, 128], BF16, tag="wfc1b")
                nc.sync.dma_start(t_w1b[:, :], wfc1b[s, :32, :])
                t_w1c = wp.tile([1, 128], BF16, tag="wfc1c")
                nc.sync.dma_start(t_w1c[:, :], wfc1b[s, 32:33, :])

                # conv: psum [3*cap, 128] over 18 K-chunks + bias row
                pt = ps.tile([3 * cap, 128], F32, tag="psco")
                for k in range(18):
                    base = k * 3 * NTOT + 3 * pos0[s]
                    lhsT = t_p[:, base:base + 3 * cap]
                    nc.tensor.matmul(pt[:, :], lhsT, t_w[:, k * 128:(k + 1) * 128],
                                     start=(k == 0), stop=False)
                nc.tensor.matmul(pt[:, :], t_ones[:1, :3 * cap],
                                 t_w[:1, 18 * 128:19 * 128], start=False, stop=True)
                # lrelu -> bf16
                t_co = sb.tile([3 * 42, 128], BF16, tag="co")
                nc.scalar.activation(t_co[:3 * cap, :], pt[:, :], ACT.Prelu,
                                     alpha=t_alpha[:3 * cap, :1])
                # transpose -> [128, 3cap], then pool along free dim
                ptt = ps.tile([128, 3 * 42], BF16, tag="pst")
                nc.tensor.transpose(ptt[:, :3 * cap], t_co[:3 * cap, :],
                                    t_id[:3 * cap, :3 * cap])
                t_coT = sb.tile([128, 3 * 42], BF16, tag="coT")
                nc.vector.tensor_copy(t_coT[:, :3 * cap], ptt[:, :3 * cap])
                # pooled sum over the 3 windows (1/3 folded into wfc1 host-side)
                t_pool = sb.tile([128, 42], BF16, tag="pool")
                nc.vector.tensor_tensor(t_pool[:, :cap], t_coT[:, :cap],
                                        t_coT[:, cap:2 * cap], op=ALU.add)
                nc.vector.tensor_tensor(t_pool[:, :cap], t_pool[:, :cap],
                                        t_coT[:, 2 * cap:3 * cap], op=ALU.add)

                # fc1: psum [cap, 128]
                pt1 = ps.tile([42, 128], F32, tag="psfc1")
                nc.tensor.matmul(pt1[:cap, :], t_pool[:, :cap], t_w1a[:, :],
                                 start=True, stop=False)
                nc.tensor.matmul(pt1[:cap, :], t_char[:, pos0[s]:pos0[s] + cap],
                                 t_w1b[:32, :], start=False, stop=False)
                nc.tensor.matmul(pt1[:cap, :], t_ones[:1, :cap],
                                 t_w1c[:1, :], start=False, stop=True)
                t_h1s = sb.tile([42, 128], F32, tag="h1s")
                nc.scalar.activation(t_h1s[:cap, :], pt1[:cap, :], ACT.Relu)
                # fc2 (slot-local, DVE)
                t_w2s = sb.tile([42, 128], F32, tag="w2s")
                nc.sync.dma_start(t_w2s[:, :], w2rep[s, :, :])
                t_b2s = sb.tile([42, 1], F32, tag="b2s")
                nc.sync.dma_start(t_b2s[:, :], b2rep[s, :, :])
                t_prod = sb.tile([42, 128], F32, tag="prod")
                nc.vector.tensor_tensor(t_prod[:cap, :], t_h1s[:cap, :],
                                        t_w2s[:cap, :], op=ALU.mult)
                t_pcs = sb.tile([42, 1], F32, tag="pcs")
                nc.vector.tensor_reduce(t_pcs[:cap, :], t_prod[:cap, :],
                                        op=ALU.add, axis=mybir.AxisListType.X)
                nc.vector.tensor_tensor(t_pcs[:cap, :], t_pcs[:cap, :],
                                        t_b2s[:cap, :], op=ALU.add)
                nc.sync.dma_start(pchar_out[pos0[s]:pos0[s] + cap, :],
                                  t_pcs[:cap, :])

    return nc


# ---------------------------------------------------------------------------
# Host-side prep
# ---------------------------------------------------------------------------

def _edge_mask_win(lv, q):
    w_, hh, (mm, oo), ext, win = GEOM[lv]
    lo = mm * q + oo
    m = np.ones(w_, np.float32)
    js = np.arange(w_)
    m[(lo + js < 0) | (lo + js >= ext)] = 0.0
    return np.concatenate([m[:win], m[w_ - win:]])


def host_prep_backbone(inp):
    """Returns list of 8 input dicts."""
    x = np.asarray(inp['x'])
    g_style = np.asarray(inp['g_style'])
    sp1_w, sp2_w = np.asarray(inp['sp1_w']), np.asarray(inp['sp2_w'])
    in_w, in_b = np.asarray(inp['in_w']), np.asarray(inp['in_b'])
    c1w1, c1b1 = np.asarray(inp['c1w1']), np.asarray(inp['c1b1'])
    c1w2, c1b2 = np.asarray(inp['c1w2']), np.asarray(inp['c1b2'])
    c2w, c2b = np.asarray(inp['c2w']), np.asarray(inp['c2b'])
    c3w1, c3b1 = np.asarray(inp['c3w1']), np.asarray(inp['c3b1'])
    c3w2, c3b2 = np.asarray(inp['c3w2']), np.asarray(inp['c3b2'])
    fm_w, fm_b = np.asarray(inp['fm_w']), np.asarray(inp['fm_b'])
    lab = np.asarray(inp['label'])
    sc = np.asarray(inp['spaced_style'])

    # shared (core-independent) weight tensors
    w1 = np.ascontiguousarray(in_w[:, 0].reshape(64, 49).T).astype(BF)  # [49, 64]
    b1 = in_b[:, None]
    w2 = np.concatenate([c1w1[:, :64, dh, dw].T for dh in range(3) for dw in range(3)], axis=1)
    w3 = np.concatenate([0.25 * c1w2[:, :, dh, dw].T for dh in range(3) for dw in range(3)], axis=1)
    b3 = c1b2[:, None]
    w4 = np.concatenate([c2w[:, :128, dh, dw].T for dh in range(3) for dw in range(3)], axis=1)
    w5 = np.concatenate([0.25 * c3w1[:, :128, dh, dw].T for dh in range(3) for dw in range(3)], axis=1)
    b5 = c3b1[:, None]
    wsal = np.concatenate([c3w1[:, 128:, :, dw].sum(axis=2).T for dw in range(3)], axis=1)  # [112, 3*128]
    w6 = np.concatenate([0.25 * c3w2[cb * 128:(cb + 1) * 128, :, dh, dw].T
                         for dh in range(3) for dw in range(3)
                         for cb in range(2)], axis=1)                 # [128, 18*128]
    b6 = np.stack([c3b2[:128], c3b2[128:]], axis=1)                   # [128, 2]
    wfm = np.stack([fm_w[0, cb * 128:(cb + 1) * 128, dh, dw]
                    for cb in range(2)
                    for dh in range(3) for dw in range(3)], axis=1)   # [128, 18]
    bfm = fm_b[:, None]

    w_sty1 = c1w1[:, 64:]
    w_sty2 = c2w[:, 128:]

    ins = []
    for c in range(N_CORES):
        b, q = divmod(c, 4)
        style1 = sp1_w @ g_style[b]
        style2 = sp2_w @ g_style[b]
        bias1 = c1b1 + np.einsum('ocij,c->o', w_sty1, style1)
        bias2 = c2b + np.einsum('ocij,c->o', w_sty2, style2)

        lo0 = 128 * q - 36
        xs = np.zeros((64, 200), np.float32)
        s_, e_ = max(0, lo0), min(512, lo0 + 200)
        xs[:, s_ - lo0:e_ - lo0] = x[b, 0][:, s_:e_]
        # im2col: x49[(dh*7+dw), (h0*194 + w')] = xs[h0+dh, w'+dw]
        x49 = np.zeros((49, 58 * 194), np.float32)
        for dh in range(7):
            for dw in range(7):
                x49[dh * 7 + dw] = xs[dh:dh + 58, dw:dw + 194].reshape(-1)

        cw2_ = np.zeros((64, 192), np.float32)
        lo2 = 128 * q - 32
        if 0 <= 0 - lo2 < 192:
            cw2_[:, 0 - lo2] = -np.einsum('oci,c->o', w_sty1[:, :, :, 0], style1)
        if 0 <= 511 - lo2 < 192:
            cw2_[:, 511 - lo2] = -np.einsum('oci,c->o', w_sty1[:, :, :, 2], style1)
        cw4_ = np.zeros((128, 92), np.float32)
        lo4 = 64 * q - 14
        if 0 <= 0 - lo4 < 92:
            cw4_[:, 0 - lo4] = -np.einsum('oci,c->o', w_sty2[:, :, :, 0], style2)
        if 0 <= 255 - lo4 < 92:
            cw4_[:, 255 - lo4] = -np.einsum('oci,c->o', w_sty2[:, :, :, 2], style2)

        sal_full = np.concatenate([sc[:, b, :], lab[:, b, :]], axis=1).T  # [112,128]
        lo5 = 32 * q - 7
        sal_ = np.zeros((112, 46), np.float32)
        s_, e_ = max(0, lo5), min(128, lo5 + 46)
        sal_[:, s_ - lo5:e_ - lo5] = sal_full[:, s_:e_]

        d = dict(x49=x49.astype(BF), w1=w1, b1=b1, w2=w2, b2=bias1[:, None], cw2=cw2_,
                 w3=w3, b3=b3, w4=w4, b4=bias2[:, None], cw4=cw4_,
                 w5=w5, b5=b5, wsal=wsal, sal=sal_, w6=w6, b6=b6,
                 wfm=wfm, bfm=bfm)
        for lv in ('h', 'g', 'm', 'g2', 'g3', 'g4'):
            p = 64 if lv in ('h', 'g') else 128
            mw = _edge_mask_win(lv, q)
            d[f"mask_{lv}"] = np.broadcast_to(mw, (p, mw.size)).copy()
        ins.append({k: np.ascontiguousarray(v) if v.dtype == BF
                    else np.ascontiguousarray(v, np.float32) for k, v in d.items()})
    return ins


def host_route_experts(inp, mL_full):
    """Build expert-phase inputs.  Returns (caps, ins_list, tokmaps)."""
    lab = np.asarray(inp['label'])
    cls = lab.argmax(axis=2).T  # [B, T]
    B, T = cls.shape
    cd_conv_w = np.asarray(inp['cd_conv_w'])
    cd_conv_b = np.asarray(inp['cd_conv_b'])
    cd_fc1_w = np.asarray(inp['cd_fc1_w'])
    cd_fc1_b = np.asarray(inp['cd_fc1_b'])
    cd_fc2_w = np.asarray(inp['cd_fc2_w'])
    cd_fc2_b = np.asarray(inp['cd_fc2_b'])
    char_style = np.asarray(inp['char_style'])

    # expert -> token list
    etok = {}
    for b in range(B):
        for t in range(T):
            etok.setdefault(int(cls[b, t]), []).append((b, t))
    experts = sorted(etok, key=lambda e: -len(etok[e]))
    # greedy bin-pack experts onto cores by token count
    S = int(np.ceil(len(experts) / N_CORES))
    core_exp = [[] for _ in range(N_CORES)]
    core_load = np.zeros(N_CORES, int)
    for e in experts:
        cands = sorted(range(N_CORES), key=lambda c: (core_load[c],))
        for c in cands:
            if len(core_exp[c]) < S:
                core_exp[c].append(e)
                core_load[c] += len(etok[e])
                break
    # capacity profile: per slot rank, max count across cores
    caps = np.zeros(S, int)
    for c in range(N_CORES):
        core_exp[c].sort(key=lambda e: -len(etok[e]))
        for i, e in enumerate(core_exp[c]):
            caps[i] = max(caps[i], len(etok[e]))
    caps = np.maximum(caps, 1)
    NTOT = int(caps.sum())
    pos0 = np.concatenate([[0], np.cumsum(caps)]).astype(int)
    NROWS = NTOT * 5

    mLp = np.pad(mL_full, ((0, 0), (0, 0), (0, 0), (2, 2)))  # [B,256,3,68]

    ins = []
    tokmaps = []
    for c in range(N_CORES):
        patches = np.zeros((128, 18, 3 * NTOT), np.float32)
        wexp = np.zeros((S, 128, 19, 128), np.float32)
        wfc1a = np.zeros((S, 128, 128), np.float32)
        wfc1b = np.zeros((S, 33, 128), np.float32)
        charT = np.zeros((32, NTOT), np.float32)
        w2rep = np.zeros((S, 42, 128), np.float32)
        b2rep = np.zeros((S, 42, 1), np.float32)
        tokmap = []  # (canonical_pos, b, t)
        for si, e in enumerate(core_exp[c]):
            toks = etok[e]
            We = cd_conv_w[e]  # [128, 256, 3, 3]
            # wexp[k=(r*3+s3)*2+cb] = We[:, cb*128:(cb+1)*128, r, s3].T
            for rr in range(3):
                for s3 in range(3):
                    for cb in range(2):
                        k = (rr * 3 + s3) * 2 + cb
                        wexp[si, :, k, :] = We[:, cb * 128:(cb + 1) * 128, rr, s3].T
            wexp[si, 0, 18, :] = cd_conv_b[e]
            wfc1a[si] = cd_fc1_w[e][:128] / 3.0
            wfc1b[si, :32] = cd_fc1_w[e][128:]
            wfc1b[si, 32] = cd_fc1_b[e]
            for j, (b, t) in enumerate(toks):
                pos = pos0[si] + j
                idx = t // 2
                blk = mLp[b, :, :, idx:idx + 5]  # [256, 3, 5]
                cap_s = int(caps[si])
                # chunk k=(r*3+s3)*2+cb; slot cols ordered (w outer, t inner)
                # to match the device psum M-order and window pooling
                for rr in range(3):
                    for s3 in range(3):
                        for cb in range(2):
                            k = (rr * 3 + s3) * 2 + cb
                            seg = blk[cb * 128:(cb + 1) * 128, rr, s3:s3 + 3]
                            for w in range(3):
                                patches[:, k, 3 * pos0[si] + w * cap_s + j] = seg[:, w]
                charT[:, pos] = char_style[b, e]
                w2rep[si, j] = cd_fc2_w[e][:, 0]
                b2rep[si, j, 0] = cd_fc2_b[e][0]
                tokmap.append((pos, b, t))
        ins.append(dict(
            patches=np.ascontiguousarray(
                patches.reshape(128, -1).astype(BF)),
            wexp=np.ascontiguousarray(wexp.reshape(S, 128, 19 * 128).astype(BF)),
            wfc1a=np.ascontiguousarray(wfc1a.astype(BF)),
            wfc1b=np.ascontiguousarray(wfc1b.astype(BF)),
            charT=np.ascontiguousarray(charT.astype(BF)),
            w2rep=w2rep, b2rep=b2rep))
        tokmaps.append(tokmap)
    return caps, ins, tokmaps


# ---------------------------------------------------------------------------
# Entry point
# ---------------------------------------------------------------------------

def kernel(**inputs):
    inp = {k: np.asarray(v) for k, v in inputs.items()}
    B, T = 2, 128

    nc1 = build_backbone()
    ins1 = host_prep_backbone(inp)
    res1 = run_bass_kernel_spmd(nc1, ins1, list(range(N_CORES))).results

    mL_full = np.zeros((B, 256, 3, 64), np.float32)
    pM = np.zeros((B, 1, 1, 64), np.float32)
    for c in range(N_CORES):
        b, q = divmod(c, 4)
        mo = res1[c]["mL_out"]  # [2, 128, 3, 16]
        mL_full[b, :128, :, 16 * q:16 * q + 16] = mo[0]
        mL_full[b, 128:, :, 16 * q:16 * q + 16] = mo[1]
        pM[b, 0, 0, 16 * q:16 * q + 16] = res1[c]["pM_out"][0]

    caps, ins2, tokmaps = host_route_experts(inp, mL_full)
    nc2 = build_expert(caps)
    res2 = run_bass_kernel_spmd(nc2, ins2, list(range(N_CORES))).results

    pChar = np.zeros((B * T, 1), np.float32)
    for c in range(N_CORES):
        pc = res2[c]["pchar_out"]
        for (pos, b, t) in tokmaps[c]:
            pChar[b * T + t, 0] = pc[pos, 0]

    return pM, pChar
